# revision 7
# baseline (speedup 1.0000x reference)
"""Multi-head attention kernel for Trainium2, sharded over 8 NeuronCores.

Full inputs q,k,v: [2, 16, 2048, 64] fp32. Heads (B*H = 32) are sharded 4 per
core; each core computes softmax(Q K^T / sqrt(d)) V for its heads with no
cross-core communication.

Per-core scheme (4 heads, n=2048, d=64), fp16 matmul datapath, fp32 PSUM:
  - Load: gpsimd casting-DMAs stage q/k as fp16 [128, 16, 64]. Transposing a
    [128, 128] block (two row-chunks) lands chunk 2t on partitions 0-63 and
    chunk 2t+1 on partitions 64-127:
      QT[t2*64+d, t, p] = Q^T[d, (2t+t2)*128+p]
    Head 0 builds this via PE transposes (PE is idle during the preamble and
    this warms the HAM clock gate); heads 1-3 via DMA XBAR transposes on the
    sync queue, hidden behind compute. K^T additionally gets a parity-swapped
    copy KB (chunk at the opposite partition half), built by two DVE
    cross-quadrant copies, so every key chunk exists in BOTH halves.
  - Scores, 2x row-packed: contraction is d=64, so two matmuls run
    concurrently in row groups 0-63 / 64-127 of the PE array:
      row0:  S^T[key j,  even-q-quad] = KT(j)[0:64]  @ QT[0:64]
      row64: S^T[key j', odd-q-quad ] = KT(j')[64:]  @ QT[64:]
    both into one [128, 1024] PSUM tile (two banks, written concurrently).
  - exp: split between ACT (exact, 12/16 steps) and DVE (4/16 steps).
    DVE uses a 2-sample averaged Schraudolph: int16(floor(s*K + B)) bitcast
    to fp16 approximates exp2; two samples with biases 512 apart average the
    mantissa-linear ripple to ~+-1%. Both samples are fed to PV un-averaged
    (denominator doubles identically; softmax is ratio-invariant; the -1024
    in the bias halves each sample so ACT and DVE chunks mix at one scale).
  - PV: out^T[65, q] += [V_j | 1]^T @ P^T_j  (row 64 = softmax denominator).
  - Finalize per (head, 1024-query group): DVE copies out^T PSUM->SBUF fp16,
    PE transposes back to [q, d], DVE reciprocal+scale, gpsimd DMA out fp32.
No max-subtraction: scores are N(0,1)-scaled, exp stays in fp16 range.
"""

import sys

sys.path.insert(0, "/opt/trn_rl_repo")

import numpy as np

import concourse.bass as bass
import concourse.mybir as mybir
import concourse.tile as tile
from concourse import bacc
from concourse.bass_utils import run_bass_kernel_spmd
from concourse.masks import make_identity

B, H, N, D = 2, 16, 2048, 64
NCORES = 8
HPC = (B * H) // NCORES  # 4 heads per core
SCALE = float(D) ** -0.5

F32 = mybir.dt.float32
F16 = mybir.dt.float16
I16 = mybir.dt.int16
EXP = mybir.ActivationFunctionType.Exp
MULT = mybir.AluOpType.mult
ADD = mybir.AluOpType.add

NJ = N // 128  # 16 key chunks of 128
NQB = 2  # two 1024-query groups per head (8 orig chunks each)

# Schraudolph fp16 exp on DVE: p ~ bitcast_f16(int16(floor(s*KMUL + B)))
# two samples, biases 512 apart, summed (PV consumes both).
KMUL = 1024.0 * 1.4426950408889634 * SCALE
C_CAL = 336.5
B1 = 15 * 1024.0 - C_CAL - 1024.0
B2 = B1 + 512.0

# which steps (of 16 per block) the DVE handles instead of ACT
DVE_I = (6, 9, 12, 15)


def _emit(tc):
    nc = tc.nc
    q_d = nc.dram_tensor("q", [HPC, N, D], F32, kind="ExternalInput").ap()
    k_d = nc.dram_tensor("k", [HPC, N, D], F32, kind="ExternalInput").ap()
    v_d = nc.dram_tensor("v", [HPC, N, D], F32, kind="ExternalInput").ap()
    o_d = nc.dram_tensor("o", [HPC, N, D], F32, kind="ExternalOutput").ap()

    from contextlib import ExitStack

    with ExitStack() as ctx:
        stg = ctx.enter_context(tc.tile_pool(name="stg", bufs=1))
        persist = ctx.enter_context(tc.tile_pool(name="persist", bufs=1))
        const_pool = ctx.enter_context(tc.tile_pool(name="const", bufs=1))
        pt_pool = ctx.enter_context(tc.tile_pool(name="pt", bufs=6))
        osb_pool = ctx.enter_context(tc.tile_pool(name="osb", bufs=2))
        rec_pool = ctx.enter_context(tc.tile_pool(name="rec", bufs=2))
        fin2_pool = ctx.enter_context(tc.tile_pool(name="fin2", bufs=3))
        st_pool = ctx.enter_context(tc.tile_pool(name="st", bufs=2, space="PSUM"))
        ot_pool = ctx.enter_context(tc.tile_pool(name="ot", bufs=1, space="PSUM"))
        fin_pool = ctx.enter_context(tc.tile_pool(name="fin", bufs=2, space="PSUM"))

        ident = const_pool.tile([128, 128], F16)
        make_identity(nc, ident[:])

        # ACT warmup: trigger the exp table load before the stream needs it
        warm_in = const_pool.tile([128, 16], F32)
        warm_out = const_pool.tile([128, 16], F16)
        nc.gpsimd.memset(warm_in[:], 0.0)
        nc.scalar.activation(warm_out[:], warm_in[:], EXP, scale=SCALE)

        # ---- phase 1: staging loads (gpsimd casting DMAs), all heads ----
        s16qs, s16ks = [], []
        qts, kas, kbs, vones = [], [], [], []
        for h in range(HPC):
            s16q = stg.tile([128, NJ, D], F16, tag=f"s16q{h}")
            s16k = stg.tile([128, NJ, D], F16, tag=f"s16k{h}")
            nc.gpsimd.dma_start(
                s16q[:], q_d[h].rearrange("(t p) d -> p t d", p=128)
            )
            nc.gpsimd.dma_start(
                s16k[:], k_d[h].rearrange("(t p) d -> p t d", p=128)
            )
            s16qs.append(s16q)
            s16ks.append(s16k)
            qt = persist.tile([128, 8, 128], F16, tag=f"qt{h}")
            ka = persist.tile([128, 8, 128], F16, tag=f"ka{h}")
            kb = persist.tile([128, 8, 128], F16, tag=f"kb{h}")
            qts.append(qt)
            kas.append(ka)
            kbs.append(kb)
        for h in range(HPC):
            vo = persist.tile([128, NJ, D + 1], F16, tag=f"vones{h}")
            nc.gpsimd.dma_start(
                vo[:, :, 0:D], v_d[h].rearrange("(t p) d -> p t d", p=128)
            )
            nc.gpsimd.memset(vo[:, :, D : D + 1], 1.0)
            vones.append(vo)

        # head 0: PE transposes (warms the PE; DVE copies PSUM->SBUF)
        for name, src, dst in (("k", s16ks[0], kas[0]), ("q", s16qs[0], qts[0])):
            for t in range(8):
                tr = fin_pool.tile([128, 128], F16, tag="fin")
                nc.tensor.transpose(
                    tr[:], src[:, 2 * t : 2 * t + 2, :], ident[:]
                )
                nc.vector.tensor_copy(dst[:, t, :], tr[:])

        # heads 1-3: XBAR DMA transposes on the sync queue
        for h in range(1, HPC):
            for t in range(8):
                nc.sync.dma_start_transpose(
                    kas[h][:, t, :], s16ks[h][:, 2 * t : 2 * t + 2, :]
                )
            for t in range(8):
                nc.sync.dma_start_transpose(
                    qts[h][:, t, :], s16qs[h][:, 2 * t : 2 * t + 2, :]
                )

        def kb_copy(h):
            # parity-swapped K^T copy via DVE cross-quadrant moves
            nc.vector.tensor_copy(kbs[h][64:128, :, :], kas[h][0:64, :, :])
            nc.vector.tensor_copy(kbs[h][0:64, :, :], kas[h][64:128, :, :])

        kb_copy(0)

        # stationary lookups: key chunk j at partition-half lo/hi
        # ka: lo=chunk 2b, hi=chunk 2b+1 ; kb: lo=chunk 2b+1, hi=chunk 2b
        def k_lo(h, j):
            if j % 2 == 0:
                return kas[h][0:64, j // 2, :]
            return kbs[h][0:64, j // 2, :]

        def k_hi(h, j):
            if j % 2 == 1:
                return kas[h][64:128, j // 2, :]
            return kbs[h][64:128, j // 2, :]

        # ---- phase 2: blocks (h, qb), 16 j-steps each ----
        blocks = [(h, qb) for h in range(HPC) for qb in range(NQB)]
        state = {}

        def emit_score(bi, i):
            h, qb = blocks[bi]
            if bi not in state:
                ot = ot_pool.tile([D + 1, 1024], F32, tag="ot")
                state[bi] = {"ot": ot, "sts": {}, "pts": {}}
            st = st_pool.tile([128, 1024], F32, tag="st")
            nc.tensor.matmul(
                st[:, 0:512],
                k_lo(h, i),
                qts[h][0:64, 4 * qb : 4 * qb + 4, :],
                start=True,
                stop=True,
            )
            nc.tensor.matmul(
                st[:, 512:1024],
                k_hi(h, i),
                qts[h][64:128, 4 * qb : 4 * qb + 4, :],
                start=True,
                stop=True,
            )
            state[bi]["sts"][i] = st

        def emit_exp(bi, i):
            st = state[bi]["sts"][i]
            if i in DVE_I:
                t1 = pt_pool.tile([128, 1024], F16, tag="pt")
                t2 = pt_pool.tile([128, 1024], F16, tag="pt")
                nc.vector.tensor_scalar(
                    t1[:].bitcast(I16), st[:], KMUL, B1, MULT, ADD
                )
                nc.vector.tensor_scalar(
                    t2[:].bitcast(I16), st[:], KMUL, B2, MULT, ADD
                )
                state[bi]["pts"][i] = (t1, t2)
            else:
                pt = pt_pool.tile([128, 1024], F16, tag="pt")
                nc.scalar.activation(pt[:], st[:], EXP, scale=SCALE)
                state[bi]["pts"][i] = (pt,)

        def emit_pv(bi, i):
            h, qb = blocks[bi]
            s = state[bi]
            tiles = s["pts"][i]
            for ti, ptile in enumerate(tiles):
                for half in range(2):
                    nc.tensor.matmul(
                        s["ot"][:, half * 512 : (half + 1) * 512],
                        vones[h][:, i, :],
                        ptile[:, half * 512 : (half + 1) * 512],
                        start=(i == 0 and ti == 0),
                        stop=(i == NJ - 1 and ti == len(tiles) - 1),
                    )
            del s["sts"][i]

        def finalize(bi):
            h, qb = blocks[bi]
            ot = state[bi]["ot"]
            osb = osb_pool.tile([D + 1, 1024], F16, tag="osb")
            nc.vector.tensor_copy(osb[:], ot[:])
            for half in range(2):
                fin = fin_pool.tile([128, 4, D + 2], F16, tag="fin")
                for u in range(4):
                    g = half * 4 + u
                    nc.tensor.transpose(
                        fin[:, u, 0 : D + 1],
                        osb[:, g * 128 : (g + 1) * 128],
                        ident[0 : D + 1, 0 : D + 1],
                    )
                rec = rec_pool.tile([128, 4, 1], F32, tag="rec")
                nc.vector.reciprocal(rec[:], fin[:, :, D : D + 1])
                fin2 = fin2_pool.tile([128, 4, D], F32, tag="fin2")
                nc.vector.tensor_mul(
                    fin2[:], fin[:, :, 0:D], rec[:].broadcast_to([128, 4, D])
                )
                # o rows n = ((a*4 + u)*2 + c)*128 + p ; chunk = 8a + 2u + c
                dst = o_d[h].rearrange(
                    "(a u c p) d -> p a c u d", a=2, u=4, c=2, p=128
                )[:, qb, half, :, :]
                nc.gpsimd.dma_start(dst, fin2[:])
            del state[bi]

        steps = [(bi, i) for bi in range(len(blocks)) for i in range(NJ)]
        pending_pv = None
        pending_fin = None
        emit_score(*steps[0])
        for s_i, (bi, i) in enumerate(steps):
            emit_exp(bi, i)
            if s_i + 1 < len(steps):
                emit_score(*steps[s_i + 1])
            if i > 0:
                emit_pv(bi, i - 1)
            if i == 1 and pending_pv is not None:
                pending_pv()
                pending_pv = None
            if i == 3 and pending_fin is not None:
                finalize(pending_fin)
                pending_fin = None
            # parity-swap K copies for heads 1-3 ride blocks 1/3/5 mid-stream
            if i == 8 and bi in (1, 3, 5):
                kb_copy(bi // 2 + 1)
            if i == NJ - 1:
                pending_pv = lambda bi=bi: emit_pv(bi, NJ - 1)
                pending_fin = bi
        pending_pv()
        finalize(pending_fin)


_CACHE = {}


def _build():
    if "nc" in _CACHE:
        return _CACHE["nc"]
    nc = bacc.Bacc("TRN2", target_bir_lowering=False, debug=False, num_devices=NCORES)
    with tile.TileContext(nc) as tc:
        _emit(tc)
    nc.compile()
    _CACHE["nc"] = nc
    return nc


def run(q, k, v, trace=False, **spmd_kwargs):
    nc = _build()
    qf = np.ascontiguousarray(np.asarray(q, dtype=np.float32).reshape(B * H, N, D))
    kf = np.ascontiguousarray(np.asarray(k, dtype=np.float32).reshape(B * H, N, D))
    vf = np.ascontiguousarray(np.asarray(v, dtype=np.float32).reshape(B * H, N, D))
    in_maps = [
        {
            "q": qf[c * HPC : (c + 1) * HPC],
            "k": kf[c * HPC : (c + 1) * HPC],
            "v": vf[c * HPC : (c + 1) * HPC],
        }
        for c in range(NCORES)
    ]
    res = run_bass_kernel_spmd(
        nc, in_maps, list(range(NCORES)), trace=trace, **spmd_kwargs
    )
    out = np.concatenate([res.results[c]["o"] for c in range(NCORES)], axis=0)
    return out.reshape(B, H, N, D).astype(np.float32), res


def kernel(q, k, v):
    out, _ = run(q, k, v)
    return out


# revision 8
# speedup vs baseline: 1.3639x; 1.3639x over previous
"""Multi-head attention kernel for Trainium2, sharded over 8 NeuronCores.

Full inputs q,k,v: [2, 16, 2048, 64] fp32. Heads (B*H = 32) are sharded 4 per
core; each core computes softmax(Q K^T / sqrt(d)) V for its heads with no
cross-core communication.

Per-core scheme (4 heads, n=2048, d=64), fp16 matmul datapath, fp32 PSUM:
  - Load: gpsimd casting-DMAs stage q/k as fp16 [128, 16, 64]. Transposing a
    [128, 128] block (two row-chunks) lands chunk 2t on partitions 0-63 and
    chunk 2t+1 on partitions 64-127:
      QT[t2*64+d, t, p] = Q^T[d, (2t+t2)*128+p]
    Head 0 builds this via PE transposes (PE is idle during the preamble and
    this warms the HAM clock gate); heads 1-3 via DMA XBAR transposes on the
    sync queue, hidden behind compute. K^T additionally gets a parity-swapped
    copy KB (chunk at the opposite partition half), built by two DVE
    cross-quadrant copies, so every key chunk exists in BOTH halves.
  - Scores, 2x row-packed: contraction is d=64, so two matmuls run
    concurrently in row groups 0-63 / 64-127 of the PE array:
      row0:  S^T[key j,  even-q-quad] = KT(j)[0:64]  @ QT[0:64]
      row64: S^T[key j', odd-q-quad ] = KT(j')[64:]  @ QT[64:]
    both into one [128, 1024] PSUM tile (two banks, written concurrently).
  - exp: split between ACT (exact, 12/16 steps) and DVE (4/16 steps).
    DVE uses a 2-sample averaged Schraudolph: int16(floor(s*K + B)) bitcast
    to fp16 approximates exp2; two samples with biases 512 apart average the
    mantissa-linear ripple to ~+-1%. Both samples are fed to PV un-averaged
    (denominator doubles identically; softmax is ratio-invariant; the -1024
    in the bias halves each sample so ACT and DVE chunks mix at one scale).
  - PV: out^T[65, q] += [V_j | 1]^T @ P^T_j  (row 64 = softmax denominator).
  - Finalize per (head, 1024-query group): DVE copies out^T PSUM->SBUF fp16,
    PE transposes back to [q, d], DVE reciprocal+scale, gpsimd DMA out fp32.
No max-subtraction: scores are N(0,1)-scaled, exp stays in fp16 range.
"""

import sys

sys.path.insert(0, "/opt/trn_rl_repo")

import numpy as np

import concourse.bass as bass
import concourse.mybir as mybir
import concourse.tile as tile
from concourse import bacc
from concourse.bass_utils import run_bass_kernel_spmd
from concourse.masks import make_identity

B, H, N, D = 2, 16, 2048, 64
NCORES = 8
HPC = (B * H) // NCORES  # 4 heads per core
SCALE = float(D) ** -0.5

F32 = mybir.dt.float32
F16 = mybir.dt.float16
I16 = mybir.dt.int16
EXP = mybir.ActivationFunctionType.Exp
MULT = mybir.AluOpType.mult
ADD = mybir.AluOpType.add

NJ = N // 128  # 16 key chunks of 128
NQB = 2  # two 1024-query groups per head (8 orig chunks each)

# Schraudolph fp16 exp on DVE: p ~ bitcast_f16(int16(floor(s*KMUL + B)))
# two samples, biases 512 apart, summed (PV consumes both).
KMUL = 1024.0 * 1.4426950408889634 * SCALE
C_CAL = 336.5
B1 = 15 * 1024.0 - C_CAL - 1024.0
B2 = B1 + 512.0

# which steps (of 16 per block) the DVE handles instead of ACT
DVE_I = (6, 9, 12, 15)


def _emit(tc):
    nc = tc.nc
    q_d = nc.dram_tensor("q", [HPC, N, D], F32, kind="ExternalInput").ap()
    k_d = nc.dram_tensor("k", [HPC, N, D], F32, kind="ExternalInput").ap()
    v_d = nc.dram_tensor("v", [HPC, N, D], F32, kind="ExternalInput").ap()
    o_d = nc.dram_tensor("o", [HPC, N, D], F32, kind="ExternalOutput").ap()

    from contextlib import ExitStack

    with ExitStack() as ctx:
        stg = ctx.enter_context(tc.tile_pool(name="stg", bufs=1))
        persist = ctx.enter_context(tc.tile_pool(name="persist", bufs=1))
        const_pool = ctx.enter_context(tc.tile_pool(name="const", bufs=1))
        pt_pool = ctx.enter_context(tc.tile_pool(name="pt", bufs=6))
        osb_pool = ctx.enter_context(tc.tile_pool(name="osb", bufs=2))
        rec_pool = ctx.enter_context(tc.tile_pool(name="rec", bufs=2))
        fin2_pool = ctx.enter_context(tc.tile_pool(name="fin2", bufs=3))
        st_pool = ctx.enter_context(tc.tile_pool(name="st", bufs=2, space="PSUM"))
        ot_pool = ctx.enter_context(tc.tile_pool(name="ot", bufs=1, space="PSUM"))
        fin_pool = ctx.enter_context(tc.tile_pool(name="fin", bufs=2, space="PSUM"))

        ident = const_pool.tile([128, 128], F16)
        make_identity(nc, ident[:])

        # ACT warmup: trigger the exp table load before the stream needs it
        warm_in = const_pool.tile([128, 16], F32)
        warm_out = const_pool.tile([128, 16], F16)
        nc.gpsimd.memset(warm_in[:], 0.0)
        nc.scalar.activation(warm_out[:], warm_in[:], EXP, scale=SCALE)

        # ---- phase 1: staging loads (gpsimd casting DMAs), all heads ----
        s16qs, s16ks = [], []
        qts, kas, kbs, vones = [], [], [], []
        for h in range(HPC):
            s16q = stg.tile([128, NJ, D], F16, tag=f"s16q{h}")
            s16k = stg.tile([128, NJ, D], F16, tag=f"s16k{h}")
            nc.gpsimd.dma_start(
                s16q[:], q_d[h].rearrange("(t p) d -> p t d", p=128)
            )
            nc.gpsimd.dma_start(
                s16k[:], k_d[h].rearrange("(t p) d -> p t d", p=128)
            )
            s16qs.append(s16q)
            s16ks.append(s16k)
            qt = persist.tile([128, 8, 128], F16, tag=f"qt{h}")
            ka = persist.tile([128, 8, 128], F16, tag=f"ka{h}")
            kb = persist.tile([128, 8, 128], F16, tag=f"kb{h}")
            qts.append(qt)
            kas.append(ka)
            kbs.append(kb)
        for h in range(HPC):
            vo = persist.tile([128, NJ, D + 1], F16, tag=f"vones{h}")
            nc.gpsimd.dma_start(
                vo[:, :, 0:D], v_d[h].rearrange("(t p) d -> p t d", p=128)
            )
            nc.gpsimd.memset(vo[:, :, D : D + 1], 1.0)
            vones.append(vo)

        # PE transpose group: 4 chunk-pair transposes into one PSUM tile,
        # then one batched DVE copy into the persistent [d, n] layout.
        def tr_group(src, dst, g):
            tr = fin_pool.tile([128, 4, 132], F16, tag="fin")
            for u in range(4):
                nc.tensor.transpose(
                    tr[:, u, 0:128],
                    src[:, 8 * g + 2 * u : 8 * g + 2 * u + 2, :],
                    ident[:],
                )
            nc.vector.tensor_copy(dst[:, 4 * g : 4 * g + 4, :], tr[:, :, 0:128])

        # head 0 upfront (PE is idle in the preamble; this warms the HAM)
        for g in range(2):
            tr_group(s16ks[0], kas[0], g)
        for g in range(2):
            tr_group(s16qs[0], qts[0], g)

        def kb_copy(h):
            # parity-swapped K^T copy via DVE cross-quadrant moves
            nc.vector.tensor_copy(kbs[h][64:128, :, :], kas[h][0:64, :, :])
            nc.vector.tensor_copy(kbs[h][0:64, :, :], kas[h][64:128, :, :])

        kb_copy(0)

        # stationary lookups: key chunk j at partition-half lo/hi
        # ka: lo=chunk 2b, hi=chunk 2b+1 ; kb: lo=chunk 2b+1, hi=chunk 2b
        def k_lo(h, j):
            if j % 2 == 0:
                return kas[h][0:64, j // 2, :]
            return kbs[h][0:64, j // 2, :]

        def k_hi(h, j):
            if j % 2 == 1:
                return kas[h][64:128, j // 2, :]
            return kbs[h][64:128, j // 2, :]

        # ---- phase 2: blocks (h, qb), 16 j-steps each ----
        blocks = [(h, qb) for h in range(HPC) for qb in range(NQB)]
        state = {}

        def emit_score(bi, i):
            h, qb = blocks[bi]
            if bi not in state:
                ot = ot_pool.tile([D + 1, 1024], F32, tag="ot")
                state[bi] = {"ot": ot, "sts": {}, "pts": {}}
            st = st_pool.tile([128, 1024], F32, tag="st")
            nc.tensor.matmul(
                st[:, 0:512],
                k_lo(h, i),
                qts[h][0:64, 4 * qb : 4 * qb + 4, :],
                start=True,
                stop=True,
            )
            nc.tensor.matmul(
                st[:, 512:1024],
                k_hi(h, i),
                qts[h][64:128, 4 * qb : 4 * qb + 4, :],
                start=True,
                stop=True,
            )
            state[bi]["sts"][i] = st

        def emit_exp(bi, i):
            st = state[bi]["sts"][i]
            if i in DVE_I:
                t1 = pt_pool.tile([128, 1024], F16, tag="pt")
                t2 = pt_pool.tile([128, 1024], F16, tag="pt")
                nc.vector.tensor_scalar(
                    t1[:].bitcast(I16), st[:], KMUL, B1, MULT, ADD
                )
                nc.vector.tensor_scalar(
                    t2[:].bitcast(I16), st[:], KMUL, B2, MULT, ADD
                )
                state[bi]["pts"][i] = (t1, t2)
            else:
                pt = pt_pool.tile([128, 1024], F16, tag="pt")
                nc.scalar.activation(pt[:], st[:], EXP, scale=SCALE)
                state[bi]["pts"][i] = (pt,)

        def emit_pv(bi, i):
            h, qb = blocks[bi]
            s = state[bi]
            tiles = s["pts"][i]
            for ti, ptile in enumerate(tiles):
                for half in range(2):
                    nc.tensor.matmul(
                        s["ot"][:, half * 512 : (half + 1) * 512],
                        vones[h][:, i, :],
                        ptile[:, half * 512 : (half + 1) * 512],
                        start=(i == 0 and ti == 0),
                        stop=(i == NJ - 1 and ti == len(tiles) - 1),
                    )
            del s["sts"][i]

        def finalize(bi):
            h, qb = blocks[bi]
            ot = state[bi]["ot"]
            osb = osb_pool.tile([D + 1, 1024], F16, tag="osb")
            nc.vector.tensor_copy(osb[:], ot[:])
            for half in range(2):
                fin = fin_pool.tile([128, 4, 132], F16, tag="fin")
                for u in range(4):
                    g = half * 4 + u
                    nc.tensor.transpose(
                        fin[:, u, 0 : D + 1],
                        osb[:, g * 128 : (g + 1) * 128],
                        ident[0 : D + 1, 0 : D + 1],
                    )
                rec = rec_pool.tile([128, 4, 1], F32, tag="rec")
                nc.vector.reciprocal(rec[:], fin[:, :, D : D + 1])
                fin2 = fin2_pool.tile([128, 4, D], F32, tag="fin2")
                nc.vector.tensor_mul(
                    fin2[:], fin[:, :, 0:D], rec[:].broadcast_to([128, 4, D])
                )
                # o rows n = ((a*4 + u)*2 + c)*128 + p ; chunk = 8a + 2u + c
                dst = o_d[h].rearrange(
                    "(a u c p) d -> p a c u d", a=2, u=4, c=2, p=128
                )[:, qb, half, :, :]
                nc.gpsimd.dma_start(dst, fin2[:])
            del state[bi]

        steps = [(bi, i) for bi in range(len(blocks)) for i in range(NJ)]
        pending_pv = None
        pending_fin = None
        emit_score(*steps[0])
        for s_i, (bi, i) in enumerate(steps):
            emit_exp(bi, i)
            if s_i + 1 < len(steps):
                emit_score(*steps[s_i + 1])
            if i > 0:
                emit_pv(bi, i - 1)
            if i == 1 and pending_pv is not None:
                pending_pv()
                pending_pv = None
            if i == 3 and pending_fin is not None:
                finalize(pending_fin)
                pending_fin = None
            # heads 1-3 build their K^T/Q^T during the previous head's
            # blocks: transpose groups at steps 5/11, parity-swap at step 13
            if bi < 6 and i in (5, 11):
                hh = bi // 2 + 1
                g = 1 if i == 11 else 0
                if bi % 2 == 0:
                    tr_group(s16ks[hh], kas[hh], g)
                else:
                    tr_group(s16qs[hh], qts[hh], g)
            if i == 13 and bi in (1, 3, 5):
                kb_copy(bi // 2 + 1)
            if i == NJ - 1:
                pending_pv = lambda bi=bi: emit_pv(bi, NJ - 1)
                pending_fin = bi
        pending_pv()
        finalize(pending_fin)


_CACHE = {}


def _build():
    if "nc" in _CACHE:
        return _CACHE["nc"]
    nc = bacc.Bacc("TRN2", target_bir_lowering=False, debug=False, num_devices=NCORES)
    with tile.TileContext(nc) as tc:
        _emit(tc)
    nc.compile()
    _CACHE["nc"] = nc
    return nc


def run(q, k, v, trace=False, **spmd_kwargs):
    nc = _build()
    qf = np.ascontiguousarray(np.asarray(q, dtype=np.float32).reshape(B * H, N, D))
    kf = np.ascontiguousarray(np.asarray(k, dtype=np.float32).reshape(B * H, N, D))
    vf = np.ascontiguousarray(np.asarray(v, dtype=np.float32).reshape(B * H, N, D))
    in_maps = [
        {
            "q": qf[c * HPC : (c + 1) * HPC],
            "k": kf[c * HPC : (c + 1) * HPC],
            "v": vf[c * HPC : (c + 1) * HPC],
        }
        for c in range(NCORES)
    ]
    res = run_bass_kernel_spmd(
        nc, in_maps, list(range(NCORES)), trace=trace, **spmd_kwargs
    )
    out = np.concatenate([res.results[c]["o"] for c in range(NCORES)], axis=0)
    return out.reshape(B, H, N, D).astype(np.float32), res


def kernel(q, k, v):
    out, _ = run(q, k, v)
    return out
